# revision 5
# baseline (speedup 1.0000x reference)
"""Multi-positive InfoNCE contrastive loss on 8 Trainium2 NeuronCores.

Math (matches the reference):
    x      = embeddings / ||row||                     (L2 row normalize)
    logits = (x @ x.T) / T,  diag excluded
    loss   = [ sum_i n_i * logZ_i  -  sum_{(i,j): mask_ij} logit_ij ] / B
  where logZ_i = log sum_{j != i} exp(logit_ij),  n_i = |P_i|.

Key structural ideas (vs. a direct row-sharded scheme):
  * Symmetry: logit_ij = logit_ji, so each unordered block pair of the
    8x8 block grid (blocks of 1024 rows) is computed ONCE.  Core c
    computes gram blocks (c,c), (c,c+1..c+3), and two quadrant pieces of
    (c,c+4) -- a uniform 4.5 block-equivalents per core, exact cover.
  * Row stats (sum-exp) for the computing core's rows come free from the
    activation accumulator; the mirror rows' sum-exp contributions are
    column sums of exp(G), obtained with fp8e5 DoubleRow ones-matmuls on
    the Tensor engine (output is partition-replicated; only partition 0
    is shipped to the host, which finishes the reduction).
  * The masked-logit term enters the loss only as a GLOBAL scalar, so no
    per-row masked sums are needed: the host builds a combined mask
    M = mask(a,b) + mask(b,a)^T per computed block, and one
    scalar_tensor_tensor pass (G*ls)*M with accum_out yields partial
    sums; host adds everything up.
  * Phase 1 normalizes columns of E^T: squares and a pairwise add on DVE
    (4x mode), partition-reduce via ones-matmuls, and
    rbc = 16/sqrt(r2) = exp(-0.5*ln(r2/256)) on ACT -- Ln and Exp share
    an activation table, so no table reloads anywhere in the kernel.
  * fp8(e4m3) DoubleRow gram with x_scale=16; exp emitted as fp8e5 so the
    colsum matmuls also run in DoubleRow (0.5 cycles/row).
  * Diagonal self-similarity is excluded by extracting exp(ls*g_ii) from
    the same PSUM value included in the row accumulator (exact cancel).

Per-core inputs (host-sliced, one NEFF for all cores):
  et     [128, 8, 5120] bf16 : E^T strip (p, d-tile, col) for blocks
                               c..c+3 plus the two 512-col halves of c+4
  maskm  [1024, 4608] u8     : combined masks M for the 4.5 blocks
  ones_bf [128,128] bf16, ones_e5 [128,2,128] f8e5, ident [128,128] bf16
Outputs:
  stats  [128, 8, 11] f32 : per (p, rt): 5 se-chunk accums, 5 ms-chunk
                            accums, dexp (self-term)
  colsum [1, 4096] f32    : mirror sum-exp partials for strip cols
                            1024..5120 (blocks c+1..c+3, c+4 halves)
"""

import numpy as np
import ml_dtypes

import concourse.bass as bass
import concourse.tile as tile
from concourse import bacc, mybir
from concourse.alu_op_type import AluOpType
from concourse.bass_utils import run_bass_kernel_spmd

P = 128
N_CORES = 8
TEMP_INV = 10.0
X_SCALE = 16.0
LS = TEMP_INV / (X_SCALE * X_SCALE)  # logit scale applied to raw gram

F32 = mybir.dt.float32
BF16 = mybir.dt.bfloat16
FP8 = mybir.dt.float8e4
FP8E5 = mybir.dt.float8e5
U8 = mybir.dt.uint8
DR = mybir.MatmulPerfMode.DoubleRow

AF = mybir.ActivationFunctionType

B_FULL, D_FULL = 8192, 1024
BC = B_FULL // N_CORES     # 1024 rows per block
N_RT = BC // P             # 8 row tiles per block
N_DT = D_FULL // P         # 8 d tiles
SW = 5 * BC                # 5120 strip cols
MW = 4 * BC + 512          # 4608 mask cols
GW = 512                   # phase-1 group width
N_G = SW // GW             # 10 phase-1 groups

# phase-2 chunks: (name, strip_c0, width, rts, cs_out_off or None, mask_c0)
CHUNKS = [
    ("diag", 0,    1024, range(8), None, 0),
    ("c1",   1024, 1024, range(8), 0,    1024),
    ("c2",   2048, 1024, range(8), 1024, 2048),
    ("c3",   3072, 1024, range(8), 2048, 3072),
    ("s4a",  4096, 512,  range(4), 3072, 4096),
    ("s4b",  4608, 512,  range(4, 8), 3584, 4096),
]


def _build():
    nc = bacc.Bacc(
        "TRN2", target_bir_lowering=False, debug=False, num_devices=N_CORES
    )
    et = nc.dram_tensor("et", (P, N_DT, SW), BF16, kind="ExternalInput").ap()
    maskm = nc.dram_tensor("maskm", (BC, MW), U8, kind="ExternalInput").ap()
    ones_bf = nc.dram_tensor("ones_bf", (P, P), BF16, kind="ExternalInput").ap()
    ones_e5 = nc.dram_tensor(
        "ones_e5", (P, 2, P), FP8E5, kind="ExternalInput"
    ).ap()
    ident = nc.dram_tensor("ident", (P, P), BF16, kind="ExternalInput").ap()
    stats = nc.dram_tensor(
        "stats", (P, N_RT, 11), F32, kind="ExternalOutput"
    ).ap()
    colsum = nc.dram_tensor("colsum", (1, 4096), F32, kind="ExternalOutput").ap()

    with tile.TileContext(nc) as tc:
        with tc.tile_pool(name="outer", bufs=1) as outer:
            xs = outer.tile([P, N_DT, SW], FP8, tag="xs")
            maskt = outer.tile([P, N_RT, MW], U8, tag="maskt")
            rbc = outer.tile([P, SW], BF16, tag="rbc")
            ones_sb = outer.tile([P, P], BF16, tag="ones_sb")
            onese5_sb = outer.tile([P, 2, P], FP8E5, tag="onese5_sb")
            ident_sb = outer.tile([P, P], BF16, tag="ident_sb")
            stats_sb = outer.tile([P, N_RT, 11], F32, tag="stats_sb")
            cs_sb = outer.tile([1, 4096], F32, tag="cs_sb")

            nc.sync.dma_start(ones_sb[:], ones_bf)
            nc.sync.dma_start(onese5_sb[:], ones_e5)
            nc.sync.dma_start(ident_sb[:], ident)
            for rt in range(N_RT):
                nc.sync.dma_start(
                    maskt[:, rt], maskm[rt * P : (rt + 1) * P, :]
                )

            with (
                tc.tile_pool(name="p_stage", bufs=3) as p_stage,
                tc.tile_pool(name="p_sq", bufs=2) as p_sq,
                tc.tile_pool(name="p_sq2", bufs=2) as p_sq2,
                tc.tile_pool(name="p_l", bufs=2) as p_l,
                tc.tile_pool(name="p_xsc", bufs=3) as p_xsc,
                tc.tile_pool(name="p_exp", bufs=3) as p_exp,
                tc.tile_pool(name="p_scr", bufs=4) as p_scr,
                tc.tile_pool(name="ps_r2", bufs=1, space="PSUM") as ps_r2,
                tc.tile_pool(name="ps_g", bufs=2, space="PSUM") as ps_g,
                tc.tile_pool(name="ps_cs", bufs=1, space="PSUM") as ps_cs,
            ):

                def phase1(g):
                    c0 = g * GW
                    stage = p_stage.tile([P, N_DT, GW], BF16, tag="stage")
                    nc.sync.dma_start(stage[:], et[:, :, c0 : c0 + GW])
                    sq = p_sq.tile([P, N_DT, GW], BF16, tag="sq")
                    nc.vector.tensor_mul(sq[:], stage[:], stage[:])
                    sq2 = p_sq2.tile([P, N_DT // 2, GW], BF16, tag="sq2")
                    nc.vector.tensor_add(
                        sq2[:], sq[:, : N_DT // 2], sq[:, N_DT // 2 :]
                    )
                    r2 = ps_r2.tile([P, GW], F32, tag="r2")
                    for k in range(N_DT // 2):
                        nc.tensor.matmul(
                            r2[:],
                            ones_sb[:],
                            sq2[:, k],
                            start=(k == 0),
                            stop=(k == N_DT // 2 - 1),
                        )
                    # rbc = X_SCALE / sqrt(r2) = exp(-0.5 * ln(r2 / X_SCALE^2))
                    # (Ln and Exp share an activation table -> no reloads)
                    lbuf = p_l.tile([P, GW], F32, tag="lbuf")
                    nc.scalar.activation(
                        lbuf[:], r2[:], AF.Ln, scale=float(1.0 / (X_SCALE**2))
                    )
                    nc.scalar.activation(
                        rbc[:, c0 : c0 + GW], lbuf[:], AF.Exp, scale=-0.5
                    )
                    xsc = p_xsc.tile([P, N_DT, GW], BF16, tag="xsc")
                    for dt in range(N_DT):
                        nc.vector.tensor_mul(
                            xsc[:, dt], stage[:, dt], rbc[:, c0 : c0 + GW]
                        )
                    nc.gpsimd.dma_start(xs[:, :, c0 : c0 + GW], xsc[:])

                def phase2(ci):
                    name, c0, w, rts, cs_off, m0 = CHUNKS[ci]
                    rts = list(rts)
                    cs = None
                    if cs_off is not None:
                        cs = ps_cs.tile([P, 1024], F32, tag="cs")
                    ex = None
                    for idx, rt in enumerate(rts):
                        gps = ps_g.tile([P, 1024], F32, tag="gps")
                        for h0 in range(0, w, 512):
                            for dt in range(0, N_DT, 2):
                                nc.tensor.matmul(
                                    gps[:, h0 : h0 + 512],
                                    xs[:, dt : dt + 2, rt * P : (rt + 1) * P],
                                    xs[:, dt : dt + 2, c0 + h0 : c0 + h0 + 512],
                                    start=(dt == 0),
                                    stop=(dt == N_DT - 2),
                                    perf_mode=DR,
                                )
                        pair = idx % 2
                        if pair == 0:
                            ex = p_exp.tile([P, 2, 1024], FP8E5, tag="ex")
                        nc.scalar.activation(
                            ex[:, pair, :w],
                            gps[:, :w],
                            AF.Exp,
                            scale=float(LS),
                            accum_out=stats_sb[:, rt, ci5(ci) : ci5(ci) + 1],
                        )
                        scr = p_scr.tile([P, 1024], BF16, tag="scr")
                        nc.vector.scalar_tensor_tensor(
                            out=scr[:, :w],
                            in0=gps[:, :w],
                            scalar=float(LS),
                            in1=maskt[:, rt, m0 : m0 + w],
                            op0=AluOpType.mult,
                            op1=AluOpType.mult,
                            accum_out=stats_sb[:, rt, 5 + ci5(ci) : 6 + ci5(ci)],
                        )
                        if name == "diag":
                            scr2 = p_scr.tile([P, P], BF16, tag="scr2")
                            simii = p_l.tile([P, 1], F32, tag="simii")
                            nc.vector.scalar_tensor_tensor(
                                out=scr2[:],
                                in0=gps[:, rt * P : (rt + 1) * P],
                                scalar=1.0,
                                in1=ident_sb[:],
                                op0=AluOpType.mult,
                                op1=AluOpType.mult,
                                accum_out=simii[:],
                            )
                            nc.scalar.activation(
                                stats_sb[:, rt, 10:11],
                                simii[:],
                                AF.Exp,
                                scale=float(LS),
                            )
                        if cs is not None and pair == 1:
                            for h0 in range(0, w, 512):
                                nc.tensor.matmul(
                                    cs[:, h0 : h0 + 512],
                                    onese5_sb[:],
                                    ex[:, :, h0 : h0 + 512],
                                    start=(idx == 1),
                                    stop=(idx == len(rts) - 1),
                                    perf_mode=DR,
                                )
                    if cs is not None:
                        nc.vector.tensor_copy(
                            cs_sb[0:1, cs_off : cs_off + w], cs[0:1, :w]
                        )

                # interleave: stream phase-1 groups into phase-2 chunks
                phase1(0)
                phase1(1)
                phase2(0)
                phase1(2)
                phase1(3)
                phase2(1)
                phase1(4)
                phase1(5)
                phase2(2)
                phase1(6)
                phase1(7)
                phase2(3)
                phase1(8)
                phase2(4)
                phase1(9)
                phase2(5)

            nc.sync.dma_start(stats, stats_sb[:])
            nc.sync.dma_start(colsum, cs_sb[:])

    nc.compile()
    return nc


def ci5(ci):
    """stats chunk slot: diag,c1,c2,c3 -> 0..3; s4a/s4b -> 4."""
    return min(ci, 4)


_CACHE = {}


def _get_nc(*a, **k):
    if "nc" not in _CACHE:
        _CACHE["nc"] = _build()
    return _CACHE["nc"]


def _halves(c):
    h1 = slice(0, 512) if c < 4 else slice(512, 1024)
    h2 = slice(512, 1024) if c < 4 else slice(0, 512)
    return h1, h2


def _run(embeddings, positives_mask, trace=False):
    B, D = embeddings.shape
    assert (B, D) == (B_FULL, D_FULL)
    nc = _get_nc()

    et_f = np.ascontiguousarray(embeddings.T).astype(ml_dtypes.bfloat16)
    # [D, B] -> [p, dt, col]
    et_p = et_f.reshape(N_DT, P, B).transpose(1, 0, 2)
    mk = positives_mask.astype(np.uint8)

    ones_bf = np.ones((P, P), dtype=ml_dtypes.bfloat16)
    ones_e5 = np.ones((P, 2, P), dtype=ml_dtypes.float8_e5m2)
    ident = np.eye(P, dtype=ml_dtypes.bfloat16)

    in_maps = []
    for c in range(N_CORES):
        h1, h2 = _halves(c)
        blk = lambda k: slice(((c + k) % 8) * BC, ((c + k) % 8) * BC + BC)
        c4 = blk(4)
        strip = np.concatenate(
            [et_p[:, :, blk(k)] for k in range(4)]
            + [
                et_p[:, :, c4][:, :, h1],
                et_p[:, :, c4][:, :, h2],
            ],
            axis=2,
        )
        R = blk(0)
        mparts = [mk[R, R]]
        for k in (1, 2, 3):
            C = blk(k)
            mparts.append(mk[R, C] + mk[C, R].T)
        d4 = np.empty((BC, 512), dtype=np.uint8)
        Rt, Rb = slice(R.start, R.start + 512), slice(R.start + 512, R.stop)
        C4h1 = slice(c4.start + h1.start, c4.start + h1.stop)
        C4h2 = slice(c4.start + h2.start, c4.start + h2.stop)
        d4[:512] = mk[Rt, C4h1] + mk[C4h1, Rt].T
        d4[512:] = mk[Rb, C4h2] + mk[C4h2, Rb].T
        mparts.append(d4)
        maskm = np.ascontiguousarray(np.concatenate(mparts, axis=1))
        in_maps.append(
            {
                "et": np.ascontiguousarray(strip),
                "maskm": maskm,
                "ones_bf": ones_bf,
                "ones_e5": ones_e5,
                "ident": ident,
            }
        )

    res = run_bass_kernel_spmd(
        nc, in_maps, core_ids=list(range(N_CORES)), trace=trace
    )

    # ---- host reduction (float64) ----
    sumexp = np.zeros(B, dtype=np.float64)
    masked = np.float64(0.0)
    for c in range(N_CORES):
        st = res.results[c]["stats"].astype(np.float64)  # [128, 8, 11]
        cs = res.results[c]["colsum"].astype(np.float64).reshape(-1)
        se_direct = st[:, :, 0:5].sum(axis=2) - st[:, :, 10]  # [128, 8]
        rows = c * BC + np.arange(BC)
        sumexp[rows] += se_direct.T.reshape(-1)
        masked += st[:, :, 5:10].sum()
        h1, h2 = _halves(c)
        for k, off in ((1, 0), (2, 1024), (3, 2048)):
            rows_k = ((c + k) % 8) * BC + np.arange(BC)
            sumexp[rows_k] += cs[off : off + BC]
        c4base = ((c + 4) % 8) * BC
        sumexp[c4base + np.arange(h1.start, h1.stop)] += cs[3072:3584]
        sumexp[c4base + np.arange(h2.start, h2.stop)] += cs[3584:4096]

    n_all = positives_mask.sum(axis=1, dtype=np.int64).astype(np.float64)
    loss = (np.sum(n_all * np.log(sumexp)) - masked) / B
    return np.float32(loss), res


def kernel(embeddings, positives_mask):
    loss, _ = _run(
        np.asarray(embeddings, dtype=np.float32),
        np.asarray(positives_mask),
    )
    return loss


# revision 10
# speedup vs baseline: 1.1051x; 1.1051x over previous
"""Multi-positive InfoNCE contrastive loss on 8 Trainium2 NeuronCores.

Math (matches the reference):
    x      = embeddings / ||row||                     (L2 row normalize)
    logits = (x @ x.T) / T,  diag excluded
    loss   = [ sum_i n_i * logZ_i  -  sum_{(i,j): mask_ij} logit_ij ] / B
  where logZ_i = log sum_{j != i} exp(logit_ij),  n_i = |P_i|.

Key structural ideas (vs. a direct row-sharded scheme):
  * Symmetry: logit_ij = logit_ji, so each unordered block pair of the
    8x8 block grid (blocks of 1024 rows) is computed ONCE.  Core c
    computes gram blocks (c,c), (c,c+1..c+3), and two quadrant pieces of
    (c,c+4) -- a uniform 4.5 block-equivalents per core, exact cover.
  * Row stats (sum-exp) for the computing core's rows come free from the
    activation accumulator; the mirror rows' sum-exp contributions are
    column sums of exp(G), obtained with fp8e5 DoubleRow ones-matmuls on
    the Tensor engine (output is partition-replicated; only partition 0
    is shipped to the host, which finishes the reduction).
  * The masked-logit term enters the loss only as a GLOBAL scalar, so no
    per-row masked sums are needed: the host builds a combined mask
    M = mask(a,b) + mask(b,a)^T per computed block, and one
    scalar_tensor_tensor pass (G*ls)*M with accum_out yields partial
    sums; host adds everything up.
  * Phase 1 normalizes columns of E^T: squares and a pairwise add on DVE
    (4x mode), partition-reduce via ones-matmuls, and
    rbc = 16/sqrt(r2) = exp(-0.5*ln(r2/256)) on ACT -- Ln and Exp share
    an activation table, so no table reloads anywhere in the kernel.
  * fp8(e4m3) DoubleRow gram with x_scale=16; exp emitted as fp8e5 so the
    colsum matmuls also run in DoubleRow (0.5 cycles/row).
  * Diagonal self-similarity is excluded by extracting exp(ls*g_ii) from
    the same PSUM value included in the row accumulator (exact cancel).

Per-core inputs (host-sliced, one NEFF for all cores):
  et     [128, 8, 5120] bf16 : E^T strip (p, d-tile, col) for blocks
                               c..c+3 plus the two 512-col halves of c+4
  maskm  [1024, 4608] u8     : combined masks M for the 4.5 blocks
  ones_bf [128,128] bf16, ones_e5 [128,2,128] f8e5, ident [128,128] bf16
Outputs:
  stats  [128, 8, 11] f32 : per (p, rt): 5 se-chunk accums, 5 ms-chunk
                            accums, dexp (self-term)
  colsum [1, 4096] f32    : mirror sum-exp partials for strip cols
                            1024..5120 (blocks c+1..c+3, c+4 halves)
"""

import numpy as np
import ml_dtypes

import concourse.bass as bass
import concourse.tile as tile
from concourse import bacc, mybir
from concourse.alu_op_type import AluOpType
from concourse.bass_utils import run_bass_kernel_spmd

P = 128
N_CORES = 8
TEMP_INV = 10.0
X_SCALE = 16.0
LS = TEMP_INV / (X_SCALE * X_SCALE)  # logit scale applied to raw gram

F32 = mybir.dt.float32
BF16 = mybir.dt.bfloat16
FP8 = mybir.dt.float8e4
FP8E5 = mybir.dt.float8e5
U8 = mybir.dt.uint8
DR = mybir.MatmulPerfMode.DoubleRow

AF = mybir.ActivationFunctionType

class _Bacc(bacc.Bacc):
    """Bacc whose activation-table pass sees Exp/Ln only in the shared
    `natural_log_exp_and_others` table, so interleaved Ln (phase-1 rsqrt)
    and Exp (phase-2 softmax) activations share ONE table load instead of
    ping-ponging 20 reloads.  Table list order (and therefore the
    act_func_set_id <-> act_info.json index mapping) is unchanged; the
    chosen table really does contain both functions on hardware.
    """

    def insert_act_table_loads(self):
        import concourse.hw_specs as hw_specs

        has_activation = any(
            isinstance(i, mybir.InstActivation)
            for b in self.main_func.blocks
            for i in b.instructions
        )
        if not has_activation:
            return
        keep = {AF.Exp, AF.Ln}
        tables = []
        for name, funcs in hw_specs.get_activation_tables(self.m.arch).items():
            if name != "natural_log_exp_and_others":
                funcs = funcs - keep
            tables.append((name, funcs))
        import bass_rust

        bass_rust.insert_act_table_loads(self, tables)


B_FULL, D_FULL = 8192, 1024
BC = B_FULL // N_CORES     # 1024 rows per block
N_RT = BC // P             # 8 row tiles per block
N_DT = D_FULL // P         # 8 d tiles
SW = 5 * BC                # 5120 strip cols
MW = 4 * BC + 512          # 4608 mask cols
GW = 512                   # phase-1 group width
N_G = SW // GW             # 10 phase-1 groups

# phase-2 chunks: (name, strip_c0, width, rts, cs_out_off or None, mask_c0)
CHUNKS = [
    ("diag", 0,    1024, range(8), None, 0),
    ("c1",   1024, 1024, range(8), 0,    1024),
    ("c2",   2048, 1024, range(8), 1024, 2048),
    ("c3",   3072, 1024, range(8), 2048, 3072),
    ("s4a",  4096, 512,  range(4), 3072, 4096),
    ("s4b",  4608, 512,  range(4, 8), 3584, 4096),
]


def _build():
    nc = _Bacc(
        "TRN2", target_bir_lowering=False, debug=False, num_devices=N_CORES
    )
    et = nc.dram_tensor("et", (P, N_DT, SW), BF16, kind="ExternalInput").ap()
    maskm = nc.dram_tensor("maskm", (P, N_RT, MW), U8, kind="ExternalInput").ap()
    ones_bf = nc.dram_tensor("ones_bf", (P, P), BF16, kind="ExternalInput").ap()
    ones_e5 = nc.dram_tensor(
        "ones_e5", (P, 2, P), FP8E5, kind="ExternalInput"
    ).ap()
    ident = nc.dram_tensor("ident", (P, P), BF16, kind="ExternalInput").ap()
    stats = nc.dram_tensor(
        "stats", (P, N_RT, 11), F32, kind="ExternalOutput"
    ).ap()
    colsum = nc.dram_tensor("colsum", (1, 4096), F32, kind="ExternalOutput").ap()

    with tile.TileContext(nc) as tc:
        with tc.tile_pool(name="outer", bufs=1) as outer:
            xs = outer.tile([P, N_DT, SW], FP8, tag="xs")
            maskt = outer.tile([P, N_RT, MW], U8, tag="maskt")
            rbc = outer.tile([P, SW], BF16, tag="rbc")
            ones_sb = outer.tile([P, P], BF16, tag="ones_sb")
            onese5_sb = outer.tile([P, 2, P], FP8E5, tag="onese5_sb")
            ident_sb = outer.tile([P, P], BF16, tag="ident_sb")
            stats_sb = outer.tile([P, N_RT, 11], F32, tag="stats_sb")
            cs_sb = outer.tile([1, 4096], F32, tag="cs_sb")

            nc.sync.dma_start(ones_sb[:], ones_bf)
            nc.sync.dma_start(onese5_sb[:], ones_e5)
            nc.sync.dma_start(ident_sb[:], ident)
            nc.sync.dma_start(maskt[:], maskm)

            with (
                tc.tile_pool(name="p_stage", bufs=3) as p_stage,
                tc.tile_pool(name="p_sq", bufs=2) as p_sq,
                tc.tile_pool(name="p_sq2", bufs=2) as p_sq2,
                tc.tile_pool(name="p_l", bufs=2) as p_l,
                tc.tile_pool(name="p_xsc", bufs=3) as p_xsc,
                tc.tile_pool(name="p_exp", bufs=3) as p_exp,
                tc.tile_pool(name="p_scr", bufs=4) as p_scr,
                tc.tile_pool(name="ps_r2", bufs=1, space="PSUM") as ps_r2,
                tc.tile_pool(name="ps_g", bufs=2, space="PSUM") as ps_g,
                tc.tile_pool(name="ps_cs", bufs=1, space="PSUM") as ps_cs,
            ):

                def phase1(g):
                    c0 = g * GW
                    stage = p_stage.tile([P, N_DT, GW], BF16, tag="stage")
                    nc.sync.dma_start(stage[:], et[:, :, c0 : c0 + GW])
                    sq = p_sq.tile([P, N_DT, GW], BF16, tag="sq")
                    nc.vector.tensor_mul(sq[:], stage[:], stage[:])
                    sq2 = p_sq2.tile([P, N_DT // 2, GW], BF16, tag="sq2")
                    nc.vector.tensor_add(
                        sq2[:], sq[:, : N_DT // 2], sq[:, N_DT // 2 :]
                    )
                    r2 = ps_r2.tile([P, GW], F32, tag="r2")
                    for k in range(N_DT // 2):
                        nc.tensor.matmul(
                            r2[:],
                            ones_sb[:],
                            sq2[:, k],
                            start=(k == 0),
                            stop=(k == N_DT // 2 - 1),
                        )
                    # rbc = X_SCALE / sqrt(r2) = exp(-0.5 * ln(r2 / X_SCALE^2))
                    # (Ln and Exp share an activation table -> no reloads)
                    lbuf = p_l.tile([P, GW], F32, tag="lbuf")
                    nc.scalar.activation(
                        lbuf[:], r2[:], AF.Ln, scale=float(1.0 / (X_SCALE**2))
                    )
                    nc.scalar.activation(
                        rbc[:, c0 : c0 + GW], lbuf[:], AF.Exp, scale=-0.5
                    )
                    xsc = p_xsc.tile([P, N_DT, GW], BF16, tag="xsc")
                    for dt in range(N_DT):
                        nc.vector.tensor_mul(
                            xsc[:, dt], stage[:, dt], rbc[:, c0 : c0 + GW]
                        )
                    nc.gpsimd.dma_start(xs[:, :, c0 : c0 + GW], xsc[:])

                def phase2(ci):
                    name, c0, w, rts, cs_off, m0 = CHUNKS[ci]
                    rts = list(rts)
                    cs = None
                    if cs_off is not None:
                        cs = ps_cs.tile([P, 1024], F32, tag="cs")
                    ex = None
                    for idx, rt in enumerate(rts):
                        gps = ps_g.tile([P, 1024], F32, tag="gps")
                        for h0 in range(0, w, 512):
                            for dt in range(0, N_DT, 2):
                                nc.tensor.matmul(
                                    gps[:, h0 : h0 + 512],
                                    xs[:, dt : dt + 2, rt * P : (rt + 1) * P],
                                    xs[:, dt : dt + 2, c0 + h0 : c0 + h0 + 512],
                                    start=(dt == 0),
                                    stop=(dt == N_DT - 2),
                                    perf_mode=DR,
                                )
                        pair = idx % 2
                        if pair == 0:
                            ex = p_exp.tile([P, 2, 1024], FP8E5, tag="ex")
                        nc.scalar.activation(
                            ex[:, pair, :w],
                            gps[:, :w],
                            AF.Exp,
                            scale=float(LS),
                            accum_out=stats_sb[:, rt, ci5(ci) : ci5(ci) + 1],
                        )
                        scr = p_scr.tile([P, 1024], BF16, tag="scr")
                        nc.vector.scalar_tensor_tensor(
                            out=scr[:, :w],
                            in0=gps[:, :w],
                            scalar=float(LS),
                            in1=maskt[:, rt, m0 : m0 + w],
                            op0=AluOpType.mult,
                            op1=AluOpType.mult,
                            accum_out=stats_sb[:, rt, 5 + ci5(ci) : 6 + ci5(ci)],
                        )
                        if name == "diag":
                            scr2 = p_scr.tile([P, P], BF16, tag="scr2")
                            simii = p_l.tile([P, 1], F32, tag="simii")
                            nc.vector.scalar_tensor_tensor(
                                out=scr2[:],
                                in0=gps[:, rt * P : (rt + 1) * P],
                                scalar=1.0,
                                in1=ident_sb[:],
                                op0=AluOpType.mult,
                                op1=AluOpType.mult,
                                accum_out=simii[:],
                            )
                            nc.scalar.activation(
                                stats_sb[:, rt, 10:11],
                                simii[:],
                                AF.Exp,
                                scale=float(LS),
                            )
                        if cs is not None and pair == 1:
                            for h0 in range(0, w, 512):
                                nc.tensor.matmul(
                                    cs[:, h0 : h0 + 512],
                                    onese5_sb[:],
                                    ex[:, :, h0 : h0 + 512],
                                    start=(idx == 1),
                                    stop=(idx == len(rts) - 1),
                                    perf_mode=DR,
                                )
                    if cs is not None:
                        nc.vector.tensor_copy(
                            cs_sb[0:1, cs_off : cs_off + w], cs[0:1, :w]
                        )

                # interleave: stream phase-1 groups into phase-2 chunks
                phase1(0)
                phase1(1)
                phase2(0)
                phase1(2)
                phase1(3)
                phase2(1)
                phase1(4)
                phase1(5)
                phase2(2)
                phase1(6)
                phase1(7)
                phase2(3)
                phase1(8)
                phase2(4)
                phase1(9)
                phase2(5)

            nc.sync.dma_start(stats, stats_sb[:])
            nc.sync.dma_start(colsum, cs_sb[:])

    nc.compile()
    return nc


def ci5(ci):
    """stats chunk slot: diag,c1,c2,c3 -> 0..3; s4a/s4b -> 4."""
    return min(ci, 4)


_CACHE = {}


def _get_nc(*a, **k):
    if "nc" not in _CACHE:
        _CACHE["nc"] = _build()
    return _CACHE["nc"]


def _halves(c):
    h1 = slice(0, 512) if c < 4 else slice(512, 1024)
    h2 = slice(512, 1024) if c < 4 else slice(0, 512)
    return h1, h2


def _run(embeddings, positives_mask, trace=False):
    B, D = embeddings.shape
    assert (B, D) == (B_FULL, D_FULL)
    nc = _get_nc()

    et_f = np.ascontiguousarray(embeddings.T).astype(ml_dtypes.bfloat16)
    # [D, B] -> [p, dt, col]
    et_p = et_f.reshape(N_DT, P, B).transpose(1, 0, 2)
    mk = positives_mask.astype(np.uint8)

    ones_bf = np.ones((P, P), dtype=ml_dtypes.bfloat16)
    ones_e5 = np.ones((P, 2, P), dtype=ml_dtypes.float8_e5m2)
    ident = np.eye(P, dtype=ml_dtypes.bfloat16)

    in_maps = []
    for c in range(N_CORES):
        h1, h2 = _halves(c)
        blk = lambda k: slice(((c + k) % 8) * BC, ((c + k) % 8) * BC + BC)
        c4 = blk(4)
        strip = np.concatenate(
            [et_p[:, :, blk(k)] for k in range(4)]
            + [
                et_p[:, :, c4][:, :, h1],
                et_p[:, :, c4][:, :, h2],
            ],
            axis=2,
        )
        R = blk(0)
        mparts = [mk[R, R]]
        for k in (1, 2, 3):
            C = blk(k)
            mparts.append(mk[R, C] + mk[C, R].T)
        d4 = np.empty((BC, 512), dtype=np.uint8)
        Rt, Rb = slice(R.start, R.start + 512), slice(R.start + 512, R.stop)
        C4h1 = slice(c4.start + h1.start, c4.start + h1.stop)
        C4h2 = slice(c4.start + h2.start, c4.start + h2.stop)
        d4[:512] = mk[Rt, C4h1] + mk[C4h1, Rt].T
        d4[512:] = mk[Rb, C4h2] + mk[C4h2, Rb].T
        mparts.append(d4)
        maskm = np.concatenate(mparts, axis=1)  # [1024, 4608]
        maskm = np.ascontiguousarray(
            maskm.reshape(N_RT, P, MW).transpose(1, 0, 2)
        )
        in_maps.append(
            {
                "et": np.ascontiguousarray(strip),
                "maskm": maskm,
                "ones_bf": ones_bf,
                "ones_e5": ones_e5,
                "ident": ident,
            }
        )

    res = run_bass_kernel_spmd(
        nc, in_maps, core_ids=list(range(N_CORES)), trace=trace
    )

    # ---- host reduction (float64) ----
    sumexp = np.zeros(B, dtype=np.float64)
    masked = np.float64(0.0)
    for c in range(N_CORES):
        st = res.results[c]["stats"].astype(np.float64)  # [128, 8, 11]
        cs = res.results[c]["colsum"].astype(np.float64).reshape(-1)
        se_direct = st[:, :, 0:5].sum(axis=2) - st[:, :, 10]  # [128, 8]
        rows = c * BC + np.arange(BC)
        sumexp[rows] += se_direct.T.reshape(-1)
        masked += st[:, :, 5:10].sum()
        h1, h2 = _halves(c)
        for k, off in ((1, 0), (2, 1024), (3, 2048)):
            rows_k = ((c + k) % 8) * BC + np.arange(BC)
            sumexp[rows_k] += cs[off : off + BC]
        c4base = ((c + 4) % 8) * BC
        sumexp[c4base + np.arange(h1.start, h1.stop)] += cs[3072:3584]
        sumexp[c4base + np.arange(h2.start, h2.stop)] += cs[3584:4096]

    n_all = positives_mask.sum(axis=1, dtype=np.int64).astype(np.float64)
    loss = (np.sum(n_all * np.log(sumexp)) - masked) / B
    return np.float32(loss), res


def kernel(embeddings, positives_mask):
    loss, _ = _run(
        np.asarray(embeddings, dtype=np.float32),
        np.asarray(positives_mask),
    )
    return loss


# revision 25
# speedup vs baseline: 1.6246x; 1.4701x over previous
"""Multi-positive InfoNCE contrastive loss on 8 Trainium2 NeuronCores.

Math (matches the reference):
    x      = embeddings / ||row||                     (L2 row normalize)
    logits = (x @ x.T) / T,  diag excluded
    loss   = [ sum_i n_i * logZ_i  -  sum_{(i,j): mask_ij} logit_ij ] / B
  where logZ_i = log sum_{j != i} exp(logit_ij),  n_i = |P_i|.

Key structural ideas (vs. a direct row-sharded scheme):
  * Symmetry: logit_ij = logit_ji, so each unordered block pair of the
    8x8 block grid (blocks of 1024 rows) is computed ONCE.  Core c
    computes gram blocks (c,c), (c,c+1..c+3), and two quadrant pieces of
    (c,c+4) -- a uniform 4.5 block-equivalents per core, exact cover.
  * Row stats (sum-exp) for the computing core's rows come free from the
    activation accumulator; the mirror rows' sum-exp contributions are
    column sums of exp(G), obtained with fp8e5 DoubleRow ones-matmuls on
    the Tensor engine (output is partition-replicated; only partition 0
    is shipped to the host, which finishes the reduction).
  * The masked-logit term enters the loss only as a GLOBAL scalar, so no
    per-row masked sums are needed: the host builds a combined mask
    M = mask(a,b) + mask(b,a)^T per computed block, and one
    scalar_tensor_tensor pass (G*ls)*M with accum_out yields partial
    sums; host adds everything up.
  * Phase 1 normalizes columns of E^T: squares and a pairwise add on DVE
    (4x mode), partition-reduce via ones-matmuls, and
    rbc = 16/sqrt(r2) = exp(-0.5*ln(r2/256)) on ACT -- Ln and Exp share
    an activation table, so no table reloads anywhere in the kernel.
  * fp8(e4m3) DoubleRow gram with x_scale=16; exp emitted as fp8e5 so the
    colsum matmuls also run in DoubleRow (0.5 cycles/row).
  * Diagonal self-similarity is excluded by extracting exp(ls*g_ii) from
    the same PSUM value included in the row accumulator (exact cancel).

Per-core inputs (host-sliced, one NEFF for all cores):
  et     [128, 8, 5120] bf16 : E^T strip (p, d-tile, col) for blocks
                               c..c+3 plus the two 512-col halves of c+4
  maskm  [1024, 4608] u8     : combined masks M for the 4.5 blocks
  ones_bf [128,128] bf16, ones_e5 [128,2,128] f8e5, ident [128,128] bf16
Outputs:
  stats  [128, 8, 11] f32 : per (p, rt): 5 se-chunk accums, 5 ms-chunk
                            accums, dexp (self-term)
  colsum [1, 4096] f32    : mirror sum-exp partials for strip cols
                            1024..5120 (blocks c+1..c+3, c+4 halves)
"""

import numpy as np
import ml_dtypes

import concourse.bass as bass
import concourse.tile as tile
from concourse import bacc, mybir
from concourse.alu_op_type import AluOpType
from concourse.bass_utils import run_bass_kernel_spmd

P = 128
N_CORES = 8
TEMP_INV = 10.0
X_SCALE = 16.0
LS = TEMP_INV / (X_SCALE * X_SCALE)  # logit scale applied to raw gram

F32 = mybir.dt.float32
BF16 = mybir.dt.bfloat16
FP8 = mybir.dt.float8e4
FP8E5 = mybir.dt.float8e5
U8 = mybir.dt.uint8
DR = mybir.MatmulPerfMode.DoubleRow

AF = mybir.ActivationFunctionType

class _Bacc(bacc.Bacc):
    """Bacc whose activation-table pass sees Exp/Ln only in the shared
    `natural_log_exp_and_others` table, so interleaved Ln (phase-1 rsqrt)
    and Exp (phase-2 softmax) activations share ONE table load instead of
    ping-ponging 20 reloads.  Table list order (and therefore the
    act_func_set_id <-> act_info.json index mapping) is unchanged; the
    chosen table really does contain both functions on hardware.
    """

    def insert_act_table_loads(self):
        import concourse.hw_specs as hw_specs

        has_activation = any(
            isinstance(i, mybir.InstActivation)
            for b in self.main_func.blocks
            for i in b.instructions
        )
        if not has_activation:
            return
        keep = {AF.Exp, AF.Ln}
        tables = []
        for name, funcs in hw_specs.get_activation_tables(self.m.arch).items():
            if name != "natural_log_exp_and_others":
                funcs = funcs - keep
            tables.append((name, funcs))
        import bass_rust

        bass_rust.insert_act_table_loads(self, tables)


B_FULL, D_FULL = 8192, 1024
BC = B_FULL // N_CORES     # 1024 rows per block
N_RT = BC // P             # 8 row tiles per block
N_DT = D_FULL // P         # 8 d tiles
SW = 5 * BC                # 5120 strip cols
MW = 4 * BC + 512          # 4608 mask cols
GW = 512                   # phase-1 group width
N_G = SW // GW             # 10 phase-1 groups
KG = 384                   # gathered masked-logit slots per (chunk, rt)

# phase-2 chunks: (name, strip_c0, width, rts, cs_out_off or None, mask_c0)
CHUNKS = [
    ("diag", 0,    1024, range(8), None, 0),
    ("c1",   1024, 1024, range(8), 0,    1024),
    ("c2",   2048, 1024, range(8), 1024, 2048),
    ("c3",   3072, 1024, range(8), 2048, 3072),
    ("s4a",  4096, 512,  range(4), 3072, 4096),
    ("s4b",  4608, 512,  range(4, 8), 3584, 4096),
]


def _build():
    nc = _Bacc(
        "TRN2", target_bir_lowering=False, debug=False, num_devices=N_CORES
    )
    et = nc.dram_tensor("et", (P, N_DT, SW), BF16, kind="ExternalInput").ap()
    gidx = nc.dram_tensor(
        "gidx", (P, 40, KG // 16), mybir.dt.uint16, kind="ExternalInput"
    ).ap()
    ones_bf = nc.dram_tensor("ones_bf", (P, P), BF16, kind="ExternalInput").ap()
    ones_e5 = nc.dram_tensor(
        "ones_e5", (P, 2, P), FP8E5, kind="ExternalInput"
    ).ap()
    ident = nc.dram_tensor("ident", (P, P), BF16, kind="ExternalInput").ap()
    stats = nc.dram_tensor(
        "stats", (P, N_RT, 6), F32, kind="ExternalOutput"
    ).ap()
    colsum = nc.dram_tensor("colsum", (1, 4096), F32, kind="ExternalOutput").ap()
    gath = nc.dram_tensor(
        "gath", (P, 40, KG), FP8E5, kind="ExternalOutput"
    ).ap()

    with tile.TileContext(nc) as tc:
        with tc.tile_pool(name="outer", bufs=1) as outer:
            xs = outer.tile([P, N_DT, SW], FP8, tag="xs")
            gidx_sb = outer.tile([P, 40, KG // 16], mybir.dt.uint16, tag="gidx_sb")
            gath_sb = outer.tile([P, 40, KG], FP8E5, tag="gath_sb")
            rbc = outer.tile([P, SW], BF16, tag="rbc")
            ones_sb = outer.tile([P, P], BF16, tag="ones_sb")
            onese5_sb = outer.tile([P, 2, P], FP8E5, tag="onese5_sb")
            ident_sb = outer.tile([P, P], BF16, tag="ident_sb")
            stats_sb = outer.tile([P, N_RT, 6], F32, tag="stats_sb")
            cs_sb = outer.tile([1, 4096], F32, tag="cs_sb")

            nc.sync.dma_start(ones_sb[:], ones_bf)
            nc.sync.dma_start(onese5_sb[:], ones_e5)
            nc.sync.dma_start(ident_sb[:], ident)
            nc.sync.dma_start(gidx_sb[:], gidx)

            with (
                tc.tile_pool(name="p_stage", bufs=3) as p_stage,
                tc.tile_pool(name="p_sq", bufs=2) as p_sq,
                tc.tile_pool(name="p_sq2", bufs=2) as p_sq2,
                tc.tile_pool(name="p_l", bufs=2) as p_l,
                tc.tile_pool(name="p_xsc", bufs=3) as p_xsc,
                tc.tile_pool(name="p_exp", bufs=3) as p_exp,
                tc.tile_pool(name="p_scr", bufs=4) as p_scr,
                tc.tile_pool(name="ps_r2", bufs=1, space="PSUM") as ps_r2,
                tc.tile_pool(name="ps_g", bufs=2, space="PSUM") as ps_g,
                tc.tile_pool(name="ps_cs", bufs=1, space="PSUM") as ps_cs,
            ):

                def phase1(g):
                    c0 = g * GW
                    stage = p_stage.tile([P, N_DT, GW], BF16, tag="stage")
                    nc.sync.dma_start(stage[:], et[:, :, c0 : c0 + GW])
                    # squares + pairwise add (TensorTensor: DVE 2x mode)
                    sq = p_sq.tile([P, N_DT, GW], BF16, tag="sq")
                    nc.vector.tensor_mul(sq[:], stage[:], stage[:])
                    sq2 = p_sq2.tile([P, N_DT // 2, GW], BF16, tag="sq2")
                    nc.vector.tensor_add(
                        sq2[:], sq[:, : N_DT // 2], sq[:, N_DT // 2 :]
                    )
                    r2 = ps_r2.tile([P, GW], F32, tag="r2")
                    for k in range(N_DT // 2):
                        for h0 in range(0, GW, 512):
                            nc.tensor.matmul(
                                r2[:, h0 : h0 + 512],
                                ones_sb[:],
                                sq2[:, k, h0 : h0 + 512],
                                start=(k == 0),
                                stop=(k == N_DT // 2 - 1),
                            )
                    # rbc = X_SCALE / sqrt(r2) = exp(-0.5 * ln(r2 / X_SCALE^2))
                    # (Ln and Exp share an activation table -> no reloads)
                    lbuf = p_l.tile([P, GW], F32, tag="lbuf")
                    nc.scalar.activation(
                        lbuf[:], r2[:], AF.Ln, scale=float(1.0 / (X_SCALE**2))
                    )
                    nc.scalar.activation(
                        rbc[:, c0 : c0 + GW], lbuf[:], AF.Exp, scale=-0.5
                    )
                    xsc = p_xsc.tile([P, N_DT, GW], BF16, tag="xsc")
                    rbc_b = rbc[:, c0 : c0 + GW].unsqueeze(1).broadcast_to(
                        (P, N_DT, GW)
                    )
                    nc.vector.tensor_mul(xsc[:], stage[:], rbc_b)
                    nc.gpsimd.dma_start(xs[:, :, c0 : c0 + GW], xsc[:])

                def phase2(ci):
                    name, c0, w, rts, cs_off, m0 = CHUNKS[ci]
                    rts = list(rts)
                    cs = None
                    if cs_off is not None:
                        cs = ps_cs.tile([P, 1024], F32, tag="cs")
                    ex = None
                    for idx, rt in enumerate(rts):
                        gps = ps_g.tile([P, 1024], F32, tag="gps")
                        for h0 in range(0, w, 512):
                            for dt in range(0, N_DT, 2):
                                nc.tensor.matmul(
                                    gps[:, h0 : h0 + 512],
                                    xs[:, dt : dt + 2, rt * P : (rt + 1) * P],
                                    xs[:, dt : dt + 2, c0 + h0 : c0 + h0 + 512],
                                    start=(dt == 0),
                                    stop=(dt == N_DT - 2),
                                    perf_mode=DR,
                                )
                        pair = idx % 2
                        if pair == 0:
                            ex = p_exp.tile([P, 2, 1024], FP8E5, tag="ex")
                        nc.scalar.activation(
                            ex[:, pair, :w],
                            gps[:, :w],
                            AF.Exp,
                            scale=float(LS),
                            accum_out=stats_sb[:, rt, ci5(ci) : ci5(ci) + 1],
                        )
                        # gather the exp values at the positive-mask columns
                        # (host recovers logits as ln(e5m2 exp) and reduces)
                        slot = ci5(ci) * 8 + rt
                        nc.gpsimd.indirect_copy(
                            gath_sb[:, slot],
                            ex[:, pair, :w],
                            gidx_sb[:, slot],
                            True,
                        )
                        if name == "diag":
                            scr2 = p_scr.tile([P, P], BF16, tag="scr2")
                            simii = p_l.tile([P, 1], F32, tag="simii")
                            nc.vector.scalar_tensor_tensor(
                                out=scr2[:],
                                in0=gps[:, rt * P : (rt + 1) * P],
                                scalar=1.0,
                                in1=ident_sb[:],
                                op0=AluOpType.mult,
                                op1=AluOpType.mult,
                                accum_out=simii[:],
                            )
                            nc.scalar.activation(
                                stats_sb[:, rt, 5:6],
                                simii[:],
                                AF.Exp,
                                scale=float(LS),
                            )
                        if cs is not None and pair == 1:
                            for h0 in range(0, w, 512):
                                nc.tensor.matmul(
                                    cs[:, h0 : h0 + 512],
                                    onese5_sb[:],
                                    ex[:, :, h0 : h0 + 512],
                                    start=(idx == 1),
                                    stop=(idx == len(rts) - 1),
                                    perf_mode=DR,
                                )
                    if cs is not None:
                        # colsum is partition-replicated; ship row 0 only.
                        # DVE copy (Pool cannot read PSUM).
                        nc.vector.tensor_copy(
                            cs_sb[0:1, cs_off : cs_off + w], cs[0:1, :w]
                        )

                # interleave: stream phase-1 groups into phase-2 chunks
                phase1(0)
                phase1(1)
                phase2(0)
                phase1(2)
                phase1(3)
                phase2(1)
                phase1(4)
                phase1(5)
                phase2(2)
                phase1(6)
                phase1(7)
                phase2(3)
                phase1(8)
                phase2(4)
                phase1(9)
                phase2(5)

            nc.sync.dma_start(stats, stats_sb[:])
            nc.sync.dma_start(colsum, cs_sb[:])
            nc.sync.dma_start(gath, gath_sb[:])

    nc.compile()
    return nc


def ci5(ci):
    """stats chunk slot: diag,c1,c2,c3 -> 0..3; s4a/s4b -> 4."""
    return min(ci, 4)


_CACHE = {}


def _get_nc(*a, **k):
    if "nc" not in _CACHE:
        _CACHE["nc"] = _build()
    return _CACHE["nc"]


def _halves(c):
    h1 = slice(0, 512) if c < 4 else slice(512, 1024)
    h2 = slice(512, 1024) if c < 4 else slice(0, 512)
    return h1, h2


def _core_mask_windows(mk, c):
    """Per-core combined-mask M windows: list of 5 [1024, w] u8 arrays
    (diag, c1, c2, c3, s4) in window-local column coordinates.  The s4
    window is row-split: rows 0-511 pair with block c+4 cols H1, rows
    512-1023 with cols H2."""
    h1, h2 = _halves(c)
    blk = lambda k: slice(((c + k) % 8) * BC, ((c + k) % 8) * BC + BC)
    R, c4 = blk(0), blk(4)
    wins = [mk[R, R].copy()]
    for k in (1, 2, 3):
        C = blk(k)
        wins.append(mk[R, C] + mk[C, R].T)
    d4 = np.zeros((BC, 512), dtype=np.uint8)
    Rt, Rb = slice(R.start, R.start + 512), slice(R.start + 512, R.stop)
    C4h1 = slice(c4.start + h1.start, c4.start + h1.stop)
    C4h2 = slice(c4.start + h2.start, c4.start + h2.stop)
    d4[:512] = mk[Rt, C4h1] + mk[C4h1, Rt].T
    d4[512:] = mk[Rb, C4h2] + mk[C4h2, Rb].T
    wins.append(d4)
    return wins


def _build_gather_plan(mk):
    """For every core: gidx [128, 40, KG//16] u16 (wrapped per 16-partition
    group as indirect_copy expects) plus the host-side weight info needed
    to finish the masked reduction: per slot a list over the 8 groups of
    (count, W16[16, count]) arrays."""
    plans = []
    for c in range(N_CORES):
        wins = _core_mask_windows(mk, c)
        gidx = np.zeros((P, 40, KG // 16), dtype=np.uint16)
        winfo = {}
        for ci, M in enumerate(wins):
            for rt in range(N_RT):
                slot = ci * 8 + rt
                groups = []
                for g in range(8):
                    r0 = rt * P + g * 16
                    sub = M[r0 : r0 + 16, :]  # [16, w]
                    cols = np.flatnonzero(sub.any(axis=0))
                    cnt = len(cols)
                    assert cnt <= KG, f"gather overflow {cnt}"
                    idxs = np.zeros(KG, dtype=np.uint16)
                    idxs[:cnt] = cols
                    # wrapped layout: unwrapped[i] = idx[16g + i%16, i//16]
                    gidx[g * 16 : (g + 1) * 16, slot, :] = idxs.reshape(
                        KG // 16, 16
                    ).T
                    groups.append((cnt, sub[:, cols].astype(np.float64)))
                winfo[slot] = groups
        plans.append((gidx, winfo))
    return plans


def _run(embeddings, positives_mask, trace=False):
    B, D = embeddings.shape
    assert (B, D) == (B_FULL, D_FULL)
    nc = _get_nc()

    et_f = np.ascontiguousarray(embeddings.T).astype(ml_dtypes.bfloat16)
    # [D, B] -> [p, dt, col]
    et_p = et_f.reshape(N_DT, P, B).transpose(1, 0, 2)
    mk = positives_mask.astype(np.uint8)
    plans = _build_gather_plan(mk)

    ones_bf = np.ones((P, P), dtype=ml_dtypes.bfloat16)
    ones_e5 = np.ones((P, 2, P), dtype=ml_dtypes.float8_e5m2)
    ident = np.eye(P, dtype=ml_dtypes.bfloat16)

    in_maps = []
    for c in range(N_CORES):
        h1, h2 = _halves(c)
        blk = lambda k: slice(((c + k) % 8) * BC, ((c + k) % 8) * BC + BC)
        c4 = blk(4)
        strip = np.concatenate(
            [et_p[:, :, blk(k)] for k in range(4)]
            + [
                et_p[:, :, c4][:, :, h1],
                et_p[:, :, c4][:, :, h2],
            ],
            axis=2,
        )
        in_maps.append(
            {
                "et": np.ascontiguousarray(strip),
                "gidx": plans[c][0],
                "ones_bf": ones_bf,
                "ones_e5": ones_e5,
                "ident": ident,
            }
        )

    res = run_bass_kernel_spmd(
        nc, in_maps, core_ids=list(range(N_CORES)), trace=trace
    )

    # ---- host reduction (float64) ----
    E5_MIN = np.float64(2.0 ** -16)  # smallest e5m2 subnormal (ln(0) guard)
    sumexp = np.zeros(B, dtype=np.float64)
    masked = np.float64(0.0)
    for c in range(N_CORES):
        st = res.results[c]["stats"].astype(np.float64)  # [128, 8, 6]
        cs = res.results[c]["colsum"].astype(np.float64).reshape(-1)
        gv = res.results[c]["gath"].astype(np.float64)  # [128, 40, KG]
        se_direct = st[:, :, 0:5].sum(axis=2) - st[:, :, 5]  # [128, 8]
        rows = c * BC + np.arange(BC)
        sumexp[rows] += se_direct.T.reshape(-1)
        h1, h2 = _halves(c)
        for k, off in ((1, 0), (2, 1024), (3, 2048)):
            rows_k = ((c + k) % 8) * BC + np.arange(BC)
            sumexp[rows_k] += cs[off : off + BC]
        c4base = ((c + 4) % 8) * BC
        sumexp[c4base + np.arange(h1.start, h1.stop)] += cs[3072:3584]
        sumexp[c4base + np.arange(h2.start, h2.stop)] += cs[3584:4096]
        # masked logits: ln of gathered exp values, weighted by M
        winfo = plans[c][1]
        for slot, groups in winfo.items():
            for g, (cnt, W16) in enumerate(groups):
                if cnt == 0:
                    continue
                vals = gv[g * 16 : (g + 1) * 16, slot, :cnt]
                masked += np.sum(W16 * np.log(np.maximum(vals, E5_MIN)))

    n_all = positives_mask.sum(axis=1, dtype=np.int64).astype(np.float64)
    loss = (np.sum(n_all * np.log(sumexp)) - masked) / B
    return np.float32(loss), res


def kernel(embeddings, positives_mask):
    loss, _ = _run(
        np.asarray(embeddings, dtype=np.float32),
        np.asarray(positives_mask),
    )
    return loss


# revision 57
# speedup vs baseline: 1.7139x; 1.0550x over previous
"""Multi-positive InfoNCE contrastive loss on 8 Trainium2 NeuronCores.

Math (matches the reference):
    x      = embeddings / ||row||                     (L2 row normalize)
    logits = (x @ x.T) / T,  diag excluded
    loss   = [ sum_i n_i * logZ_i  -  sum_{(i,j): mask_ij} logit_ij ] / B
  where logZ_i = log sum_{j != i} exp(logit_ij),  n_i = |P_i|.

Key structural ideas (vs. a direct row-sharded scheme):
  * Symmetry: logit_ij = logit_ji, so each unordered block pair of the
    8x8 block grid (blocks of 1024 rows) is computed ONCE.  Core c
    computes gram blocks (c,c), (c,c+1..c+3), and two quadrant pieces of
    (c,c+4) -- a uniform 4.5 block-equivalents per core, exact cover.
  * Row stats (sum-exp) for the computing core's rows come free from the
    activation accumulator; the mirror rows' sum-exp contributions are
    column sums of exp(G), obtained with fp8e5 DoubleRow ones-matmuls on
    the Tensor engine (output is partition-replicated; only partition 0
    is shipped to the host, which finishes the reduction).
  * The masked-logit term enters the loss only as a GLOBAL scalar, so no
    per-row masked sums are needed: the host builds a combined mask
    M = mask(a,b) + mask(b,a)^T per computed block, and one
    scalar_tensor_tensor pass (G*ls)*M with accum_out yields partial
    sums; host adds everything up.
  * Phase 1 normalizes columns of E^T: squares and a pairwise add on DVE
    (4x mode), partition-reduce via ones-matmuls, and
    rbc = 16/sqrt(r2) = exp(-0.5*ln(r2/256)) on ACT -- Ln and Exp share
    an activation table, so no table reloads anywhere in the kernel.
  * fp8(e4m3) DoubleRow gram with x_scale=16; exp emitted as fp8e5 so the
    colsum matmuls also run in DoubleRow (0.5 cycles/row).
  * Diagonal self-similarity is excluded by extracting exp(ls*g_ii) from
    the same PSUM value included in the row accumulator (exact cancel).

Per-core inputs (host-sliced, one NEFF for all cores):
  et     [128, 8, 5120] bf16 : E^T strip (p, d-tile, col) for blocks
                               c..c+3 plus the two 512-col halves of c+4
  maskm  [1024, 4608] u8     : combined masks M for the 4.5 blocks
  ones_bf [128,128] bf16, ones_e5 [128,2,128] f8e5, ident [128,128] bf16
Outputs:
  stats  [128, 8, 11] f32 : per (p, rt): 5 se-chunk accums, 5 ms-chunk
                            accums, dexp (self-term)
  colsum [1, 4096] f32    : mirror sum-exp partials for strip cols
                            1024..5120 (blocks c+1..c+3, c+4 halves)
"""

import numpy as np
import ml_dtypes

import concourse.bass as bass
import concourse.tile as tile
from concourse import bacc, mybir
from concourse.alu_op_type import AluOpType
from concourse.bass_utils import run_bass_kernel_spmd

P = 128
N_CORES = 8
TEMP_INV = 10.0
X_SCALE = 16.0
LS = TEMP_INV / (X_SCALE * X_SCALE)  # logit scale applied to raw gram

F32 = mybir.dt.float32
BF16 = mybir.dt.bfloat16
FP8 = mybir.dt.float8e4
FP8E5 = mybir.dt.float8e5
U8 = mybir.dt.uint8
DR = mybir.MatmulPerfMode.DoubleRow

AF = mybir.ActivationFunctionType

class _Bacc(bacc.Bacc):
    """Bacc whose activation-table pass sees Exp/Ln only in the shared
    `natural_log_exp_and_others` table, so interleaved Ln (phase-1 rsqrt)
    and Exp (phase-2 softmax) activations share ONE table load instead of
    ping-ponging 20 reloads.  Table list order (and therefore the
    act_func_set_id <-> act_info.json index mapping) is unchanged; the
    chosen table really does contain both functions on hardware.
    """

    def insert_act_table_loads(self):
        import concourse.hw_specs as hw_specs

        has_activation = any(
            isinstance(i, mybir.InstActivation)
            for b in self.main_func.blocks
            for i in b.instructions
        )
        if not has_activation:
            return
        keep = {AF.Exp, AF.Ln}
        tables = []
        for name, funcs in hw_specs.get_activation_tables(self.m.arch).items():
            if name != "natural_log_exp_and_others":
                funcs = funcs - keep
            tables.append((name, funcs))
        import bass_rust

        bass_rust.insert_act_table_loads(self, tables)


B_FULL, D_FULL = 8192, 1024
BC = B_FULL // N_CORES     # 1024 rows per block
N_RT = BC // P             # 8 row tiles per block
N_DT = D_FULL // P         # 8 d tiles
SW = 4 * BC                # 4096 strip cols
CW_OUT = 3072              # colsum output cols (c+1 | c+3 | c+4 halves)
GW = 512                   # phase-1 group width
N_G = SW // GW             # 10 phase-1 groups
KG = 384                   # gathered masked-logit slots per (chunk, rt)

# Edge assignment: core c computes gram blocks for the unordered block
# pairs (c,c), (c,c+1), (c,c+3), (c+1,c+3) and a quadrant-split half of
# (c,c+4) -- an exact, balanced cover of all 36 block pairs that needs
# only FOUR distinct embedding blocks per core: {c, c+1, c+3, c+4}.
# strip layout: [own | c4-h1 | c4-h2 | c+1 | c+3] (4096 cols).
# chunks: (name, strip_c0, width, rts, cs_out_off or None, slot_base,
#          lhsT_c0, cs_mode)  cs_mode: None, "own", "start", "stop"
# e3 and e2 accumulate their mirror colsums into ONE psum tile (both
# target block c+3's columns).
CHUNKS = [
    ("diag", 0,    1024, range(8),    None, 0,  0,    None),
    ("d4a",  1024, 512,  range(4),    2048, 8,  0,    "own"),
    ("d4b",  1536, 512,  range(4, 8), 2560, 8,  0,    "own"),
    ("e1",   2048, 1024, range(8),    0,    16, 0,    "own"),
    ("e3",   3072, 1024, range(8),    1024, 24, 0,    "start"),
    ("e2",   3072, 1024, range(8),    1024, 32, 2048, "stop"),
]


def _build():
    nc = _Bacc(
        "TRN2", target_bir_lowering=False, debug=False, num_devices=N_CORES
    )
    et = nc.dram_tensor("et", (P, N_DT, SW), BF16, kind="ExternalInput").ap()
    gidx = nc.dram_tensor(
        "gidx", (P, 40, KG // 16), mybir.dt.uint16, kind="ExternalInput"
    ).ap()
    ones_bf = nc.dram_tensor("ones_bf", (P, P), BF16, kind="ExternalInput").ap()
    ones_e5 = nc.dram_tensor(
        "ones_e5", (P, 2, P), FP8E5, kind="ExternalInput"
    ).ap()
    ident = nc.dram_tensor("ident", (P, P), BF16, kind="ExternalInput").ap()
    stats = nc.dram_tensor(
        "stats", (P, N_RT, 6), F32, kind="ExternalOutput"
    ).ap()
    colsum = nc.dram_tensor(
        "colsum", (1, CW_OUT), F32, kind="ExternalOutput"
    ).ap()
    gath = nc.dram_tensor(
        "gath", (P, 40, KG), FP8E5, kind="ExternalOutput"
    ).ap()

    with tile.TileContext(nc) as tc:
        with tc.tile_pool(name="outer", bufs=1) as outer:
            xs = outer.tile([P, N_DT, SW], FP8, tag="xs")
            gidx_sb = outer.tile([P, 40, KG // 16], mybir.dt.uint16, tag="gidx_sb")
            gath_sb = outer.tile([P, 40, KG], FP8E5, tag="gath_sb")
            rbc = outer.tile([P, SW], BF16, tag="rbc")
            ones_sb = outer.tile([P, P], BF16, tag="ones_sb")
            onese5_sb = outer.tile([P, 2, P], FP8E5, tag="onese5_sb")
            ident_sb = outer.tile([P, P], BF16, tag="ident_sb")
            stats_sb = outer.tile([P, N_RT, 6], F32, tag="stats_sb")
            cs_sb = outer.tile([1, CW_OUT], F32, tag="cs_sb")

            nc.sync.dma_start(ones_sb[:], ones_bf)
            nc.sync.dma_start(onese5_sb[:], ones_e5)
            nc.sync.dma_start(ident_sb[:], ident)
            nc.sync.dma_start(gidx_sb[:], gidx)

            with (
                tc.tile_pool(name="p_stage", bufs=3) as p_stage,
                tc.tile_pool(name="p_sq", bufs=2) as p_sq,
                tc.tile_pool(name="p_sq2", bufs=2) as p_sq2,
                tc.tile_pool(name="p_l", bufs=2) as p_l,
                tc.tile_pool(name="p_xsc", bufs=3) as p_xsc,
                tc.tile_pool(name="p_exp", bufs=3) as p_exp,
                tc.tile_pool(name="p_scr", bufs=4) as p_scr,
                tc.tile_pool(name="ps_r2", bufs=1, space="PSUM") as ps_r2,
                tc.tile_pool(name="ps_g", bufs=2, space="PSUM") as ps_g,
                tc.tile_pool(name="ps_cs", bufs=1, space="PSUM") as ps_cs,
            ):

                # phase-1 split in three pumpable steps per group so its
                # work threads through the phase-2 rt loops without ever
                # blocking an engine stream on a not-yet-ready dependency
                p1_state = {}

                def p1_dma(g):
                    c0 = g * GW
                    stage = p_stage.tile([P, N_DT, GW], BF16, tag="stage")
                    nc.sync.dma_start(stage[:], et[:, :, c0 : c0 + GW])
                    p1_state[g] = stage

                def p1_sq(g):
                    stage = p1_state[g]
                    # squares + pairwise add (TensorTensor: DVE 2x mode)
                    sq = p_sq.tile([P, N_DT, GW], BF16, tag="sq")
                    nc.vector.tensor_mul(sq[:], stage[:], stage[:])
                    sq2 = p_sq2.tile([P, N_DT // 2, GW], BF16, tag="sq2")
                    nc.vector.tensor_add(
                        sq2[:], sq[:, : N_DT // 2], sq[:, N_DT // 2 :]
                    )
                    p1_state[g] = (stage, sq2)

                def p1_fin(g):
                    c0 = g * GW
                    stage, sq2 = p1_state.pop(g)
                    r2 = ps_r2.tile([P, GW], F32, tag="r2")
                    for k in range(N_DT // 2):
                        nc.tensor.matmul(
                            r2[:],
                            ones_sb[:],
                            sq2[:, k],
                            start=(k == 0),
                            stop=(k == N_DT // 2 - 1),
                        )
                    # rbc = X_SCALE/sqrt(r2) = exp(-0.5*ln(r2/X_SCALE^2));
                    # Ln and Exp share an activation table -> no reloads
                    lbuf = p_l.tile([P, GW], F32, tag="lbuf")
                    nc.scalar.activation(
                        lbuf[:], r2[:], AF.Ln, scale=float(1.0 / (X_SCALE**2))
                    )
                    nc.scalar.activation(
                        rbc[:, c0 : c0 + GW], lbuf[:], AF.Exp, scale=-0.5
                    )
                    xsc = p_xsc.tile([P, N_DT, GW], BF16, tag="xsc")
                    rbc_b = rbc[:, c0 : c0 + GW].unsqueeze(1).broadcast_to(
                        (P, N_DT, GW)
                    )
                    nc.vector.tensor_mul(xsc[:], stage[:], rbc_b)
                    nc.gpsimd.dma_start(xs[:, :, c0 : c0 + GW], xsc[:])

                simii_all = outer.tile([P, N_RT], F32, tag="simii_all")

                cs_shared = {}

                def phase2(ci, pumps=None):
                    name, c0, w, rts, cs_off, sbase, l0, cs_mode = CHUNKS[ci]
                    sb8 = sbase // 8
                    rts = list(rts)
                    pumps = pumps or {}
                    cs = None
                    if cs_mode == "own" or cs_mode == "start":
                        cs = ps_cs.tile([P, 1024], F32, tag="cs")
                        cs_shared[0] = cs
                    elif cs_mode == "stop":
                        cs = cs_shared.pop(0)
                    cs_start = cs_mode in ("own", "start")
                    cs_stop = cs_mode in ("own", "stop")
                    ex = None
                    for idx, rt in enumerate(rts):
                        for fn in pumps.get(idx, ()):
                            fn()
                        gps = ps_g.tile([P, 1024], F32, tag="gps")
                        for h0 in range(0, w, 512):
                            for dt in range(0, N_DT, 2):
                                nc.tensor.matmul(
                                    gps[:, h0 : h0 + 512],
                                    xs[:, dt : dt + 2,
                                       l0 + rt * P : l0 + (rt + 1) * P],
                                    xs[:, dt : dt + 2, c0 + h0 : c0 + h0 + 512],
                                    start=(dt == 0),
                                    stop=(dt == N_DT - 2),
                                    perf_mode=DR,
                                )
                        pair = idx % 2
                        if pair == 0:
                            ex = p_exp.tile([P, 2, 1024], FP8E5, tag="ex")
                        nc.scalar.activation(
                            ex[:, pair, :w],
                            gps[:, :w],
                            AF.Exp,
                            scale=float(LS),
                            accum_out=stats_sb[:, rt, sb8 : sb8 + 1],
                        )
                        # gather the exp values at the positive-mask columns
                        # (host recovers logits as ln(e5m2 exp) and reduces)
                        slot = sbase + rt
                        nc.gpsimd.indirect_copy(
                            gath_sb[:, slot],
                            ex[:, pair, :w],
                            gidx_sb[:, slot],
                            True,
                        )
                        if name == "diag":
                            scr2 = p_scr.tile([P, P], BF16, tag="scr2")
                            nc.vector.scalar_tensor_tensor(
                                out=scr2[:],
                                in0=gps[:, rt * P : (rt + 1) * P],
                                scalar=1.0,
                                in1=ident_sb[:],
                                op0=AluOpType.mult,
                                op1=AluOpType.mult,
                                accum_out=simii_all[:, rt : rt + 1],
                            )
                        if cs is not None and pair == 1:
                            for h0 in range(0, w, 512):
                                nc.tensor.matmul(
                                    cs[:, h0 : h0 + 512],
                                    onese5_sb[:],
                                    ex[:, :, h0 : h0 + 512],
                                    start=(cs_start and idx == 1),
                                    stop=(cs_stop and idx == len(rts) - 1),
                                    perf_mode=DR,
                                )
                    if name == "diag":
                        # one batched self-term exp for all 8 row tiles
                        nc.scalar.activation(
                            stats_sb[:, :, 5], simii_all[:], AF.Exp,
                            scale=float(LS),
                        )
                    if cs is not None and cs_stop:
                        # colsum is partition-replicated; ship row 0 only.
                        # DVE copy (Pool cannot read PSUM).
                        nc.vector.tensor_copy(
                            cs_sb[0:1, cs_off : cs_off + w], cs[0:1, :w]
                        )
                        nc.sync.dma_start(
                            colsum[0:1, cs_off : cs_off + w],
                            cs_sb[0:1, cs_off : cs_off + w],
                        )
                    # ship this chunk's gathered values while compute continues
                    s0 = sbase + rts[0]
                    nc.sync.dma_start(
                        gath[:, s0 : s0 + len(rts)],
                        gath_sb[:, s0 : s0 + len(rts)],
                    )

                # Stream phase-1 group steps through the phase-2 rt loops.
                # Chunk needs: diag g0-1, d4a g2, d4b g3, e1 g4-5, e3 g6-7,
                # e2 nothing new.  DMAs run ~3 groups ahead, squares ~1.5,
                # finish right before the consuming chunk.
                A, B, C = p1_dma, p1_sq, p1_fin
                A(0)
                B(0)
                C(0)
                A(1)
                B(1)
                C(1)
                A(2)
                phase2(0, {0: [lambda: B(2)], 2: [lambda: C(2), lambda: A(3)],
                           4: [lambda: B(3)], 6: [lambda: C(3), lambda: A(4)]})
                phase2(1, {0: [lambda: B(4)], 1: [lambda: C(4)],
                           2: [lambda: A(5)], 3: [lambda: B(5)]})
                phase2(2, {0: [lambda: C(5)], 1: [lambda: A(6)],
                           2: [lambda: B(6)], 3: [lambda: C(6)]})
                phase2(3, {0: [lambda: A(7)], 2: [lambda: B(7)],
                           4: [lambda: C(7)]})
                phase2(4)   # e3 (g6-7 ready)
                phase2(5)   # e2 (lhsT c+1, rhs c+3 -- all ready)

            nc.sync.dma_start(stats, stats_sb[:])

    nc.compile()
    return nc


_CACHE = {}


def _get_nc(*a, **k):
    if "nc" not in _CACHE:
        _CACHE["nc"] = _build()
    return _CACHE["nc"]


def _halves(c):
    h1 = slice(0, 512) if c < 4 else slice(512, 1024)
    h2 = slice(512, 1024) if c < 4 else slice(0, 512)
    return h1, h2


def _core_mask_windows(mk, c):
    """Per-core combined-mask M windows in slot order
    (diag, d4, e1, e3, e2): [1024, w] u8 arrays, window-local column
    coords.  d4 is row-split: rows 0-511 pair with block c+4 cols H1,
    rows 512-1023 with cols H2.  e2's ROWS are block c+1 (not c)."""
    h1, h2 = _halves(c)
    blk = lambda k: slice(((c + k) % 8) * BC, ((c + k) % 8) * BC + BC)
    R, c4 = blk(0), blk(4)
    d4 = np.zeros((BC, 512), dtype=np.uint8)
    Rt, Rb = slice(R.start, R.start + 512), slice(R.start + 512, R.stop)
    C4h1 = slice(c4.start + h1.start, c4.start + h1.stop)
    C4h2 = slice(c4.start + h2.start, c4.start + h2.stop)
    d4[:512] = mk[Rt, C4h1] + mk[C4h1, Rt].T
    d4[512:] = mk[Rb, C4h2] + mk[C4h2, Rb].T
    C1, C3 = blk(1), blk(3)
    return [
        mk[R, R].copy(),
        d4,
        mk[R, C1] + mk[C1, R].T,
        mk[R, C3] + mk[C3, R].T,
        mk[C1, C3] + mk[C3, C1].T,
    ]


def _build_gather_plan(mk):
    """For every core: gidx [128, 40, KG//16] u16 (wrapped per 16-partition
    group as indirect_copy expects) plus the host-side weight info needed
    to finish the masked reduction: per slot a list over the 8 groups of
    (count, W16[16, count]) arrays."""
    plans = []
    for c in range(N_CORES):
        wins = _core_mask_windows(mk, c)
        gidx = np.zeros((P, 40, KG // 16), dtype=np.uint16)
        winfo = {}
        for ci, M in enumerate(wins):
            for rt in range(N_RT):
                slot = ci * 8 + rt
                groups = []
                for g in range(8):
                    r0 = rt * P + g * 16
                    sub = M[r0 : r0 + 16, :]  # [16, w]
                    cols = np.flatnonzero(sub.any(axis=0))
                    cnt = len(cols)
                    assert cnt <= KG, f"gather overflow {cnt}"
                    idxs = np.zeros(KG, dtype=np.uint16)
                    idxs[:cnt] = cols
                    # wrapped layout: unwrapped[i] = idx[16g + i%16, i//16]
                    gidx[g * 16 : (g + 1) * 16, slot, :] = idxs.reshape(
                        KG // 16, 16
                    ).T
                    groups.append((cnt, sub[:, cols].astype(np.float64)))
                winfo[slot] = groups
        plans.append((gidx, winfo))
    return plans


def _run(embeddings, positives_mask, trace=False):
    B, D = embeddings.shape
    assert (B, D) == (B_FULL, D_FULL)
    nc = _get_nc()

    et_f = np.ascontiguousarray(embeddings.T).astype(ml_dtypes.bfloat16)
    # [D, B] -> [p, dt, col]
    et_p = et_f.reshape(N_DT, P, B).transpose(1, 0, 2)
    mk = positives_mask.astype(np.uint8)
    plans = _build_gather_plan(mk)

    ones_bf = np.ones((P, P), dtype=ml_dtypes.bfloat16)
    ones_e5 = np.ones((P, 2, P), dtype=ml_dtypes.float8_e5m2)
    ident = np.eye(P, dtype=ml_dtypes.bfloat16)

    in_maps = []
    for c in range(N_CORES):
        h1, h2 = _halves(c)
        blk = lambda k: slice(((c + k) % 8) * BC, ((c + k) % 8) * BC + BC)
        c4 = blk(4)
        strip = np.concatenate(
            [
                et_p[:, :, blk(0)],
                et_p[:, :, c4][:, :, h1],
                et_p[:, :, c4][:, :, h2],
                et_p[:, :, blk(1)],
                et_p[:, :, blk(3)],
            ],
            axis=2,
        )
        in_maps.append(
            {
                "et": np.ascontiguousarray(strip),
                "gidx": plans[c][0],
                "ones_bf": ones_bf,
                "ones_e5": ones_e5,
                "ident": ident,
            }
        )

    res = run_bass_kernel_spmd(
        nc, in_maps, core_ids=list(range(N_CORES)), trace=trace
    )

    # ---- host reduction (float64) ----
    E5_MIN = np.float64(2.0 ** -16)  # smallest e5m2 subnormal (ln(0) guard)
    sumexp = np.zeros(B, dtype=np.float64)
    masked = np.float64(0.0)
    for c in range(N_CORES):
        st = res.results[c]["stats"].astype(np.float64)  # [128, 8, 6]
        cs = res.results[c]["colsum"].astype(np.float64).reshape(-1)
        gv = res.results[c]["gath"].astype(np.float64)  # [128, 40, KG]
        # slots 0-3 (diag, d4, e1, e3) are rows of block c; slot 4 (e2)
        # is rows of block c+1; slot 5 is the diag self-term
        se_own = st[:, :, 0:4].sum(axis=2) - st[:, :, 5]  # [128, 8]
        rows = c * BC + np.arange(BC)
        sumexp[rows] += se_own.T.reshape(-1)
        rows1 = ((c + 1) % 8) * BC + np.arange(BC)
        sumexp[rows1] += st[:, :, 4].T.reshape(-1)
        h1, h2 = _halves(c)
        sumexp[((c + 1) % 8) * BC + np.arange(BC)] += cs[0:BC]
        sumexp[((c + 3) % 8) * BC + np.arange(BC)] += cs[BC : 2 * BC]
        c4base = ((c + 4) % 8) * BC
        sumexp[c4base + np.arange(h1.start, h1.stop)] += cs[2048:2560]
        sumexp[c4base + np.arange(h2.start, h2.stop)] += cs[2560:3072]
        # masked logits: ln of gathered exp values, weighted by M
        winfo = plans[c][1]
        for slot, groups in winfo.items():
            for g, (cnt, W16) in enumerate(groups):
                if cnt == 0:
                    continue
                vals = gv[g * 16 : (g + 1) * 16, slot, :cnt]
                masked += np.sum(W16 * np.log(np.maximum(vals, E5_MIN)))

    n_all = positives_mask.sum(axis=1, dtype=np.int64).astype(np.float64)
    loss = (np.sum(n_all * np.log(sumexp)) - masked) / B
    return np.float32(loss), res


def kernel(embeddings, positives_mask):
    loss, _ = _run(
        np.asarray(embeddings, dtype=np.float32),
        np.asarray(positives_mask),
    )
    return loss
